# revision 1
# baseline (speedup 1.0000x reference)
"""Trainium2 Bass kernel for CustomLossWithCovariance.

loss = abs(logdet(sigma) + mean_b[(p_b - t_b)^T sigma^{-1} (p_b - t_b)])

Only the 3x3 Gram matrix G = sum_b d_b d_b^T (d = pred - targ) requires
touching the [B, 3] data; the device computes per-core partial pair-sums
of G, and the host finishes with the tiny 3x3 algebra:
    mean_mahalanobis = <sigma_inv, G> / B
    loss = |logdet(sigma) + mean_mahalanobis|

Sharding: data-parallel over the batch across 8 NeuronCores (each core
streams a contiguous [B/8, 3] shard; partial sums gathered on host).

Per-core device kernel (raw Bacc, manual semaphores — see
build_gram_kernel_raw; build_gram_kernel is the Tile-framework
baseline kept for reference). Per tile of [128, 2M]:
  - DMA both halves (pred | targ) flat-contiguous in one dma_start
  - DVE: d = pred - targ, in-place into the pred half (unit-stride fp32)
  - ACT: Square(d_i) with accum_out -> per-partition sums of d_i^2
  - DVE: scalar_tensor_tensor(d_i * d_j, accum_out) -> cross sums
  (component APs are stride-3 views of the flat tiles, grouped 4 tiles
  per reduce instruction to amortize fixed costs)
"""

import numpy as np

import concourse.bass as bass
import concourse.bacc as bacc
import concourse.mybir as mybir
from concourse import tile
from concourse.bass_utils import run_bass_kernel_spmd

N_CORES = 8
B_FULL = 8388608
P = 128

_PAIRS = [(0, 1), (0, 2), (1, 2)]


def build_gram_kernel(n_rows: int, n_tiles: int, use_act: bool = True):
    """Build the per-core Bass module.

    Input: pt [2, n_rows, 3] f32 (pred stacked with targ)
    Output: partials [128, 6 * n_tiles] f32
        col t*3+i            : sum over this tile/partition of d_i^2
        col 3*n_tiles + t*3+k: sum of d_i*d_j for pair k in _PAIRS
    """
    assert n_rows % (P * n_tiles) == 0
    r = n_rows // (P * n_tiles)  # rows per partition per tile
    m = 3 * r                    # flat f32 elements per partition per tile
    f32 = mybir.dt.float32

    # Bacc (not plain Bass): its compile() pass legalizes semaphore waits
    # (each TRN2 instruction holds at most one wait slot).
    nc = bacc.Bacc("TRN2", target_bir_lowering=False, debug=False)
    pt = nc.dram_tensor("pt", [2, n_rows, 3], f32, kind="ExternalInput")
    out = nc.dram_tensor("partials", [P, 6 * n_tiles], f32, kind="ExternalOutput")

    # [t][p][w(2), m] — per tile/partition: pred chunk and targ chunk, each
    # m contiguous f32 in DRAM.
    pt_v = pt[:].rearrange("w (t p r) c -> t p w (r c)", t=n_tiles, p=P)

    with tile.TileContext(nc) as tc:
        with (
            tc.tile_pool(name="io", bufs=3) as io_pool,
            tc.tile_pool(name="dve_scr", bufs=2) as dve_scr,
            tc.tile_pool(name="act_scr", bufs=2) as act_scr,
            tc.tile_pool(name="acc", bufs=1) as acc_pool,
        ):
            acc_sq = acc_pool.tile([P, 3 * n_tiles], f32)
            acc_cr = acc_pool.tile([P, 3 * n_tiles], f32)

            for t in range(n_tiles):
                buf = io_pool.tile([P, 2 * m], f32, tag="buf")
                nc.sync.dma_start(
                    out=buf[:].rearrange("p (w m) -> p w m", w=2),
                    in_=pt_v[t],
                )

                # In-place: d = pred - targ, overwriting the pred half.
                nc.vector.tensor_tensor(
                    out=buf[:, 0:m],
                    in0=buf[:, 0:m],
                    in1=buf[:, m : 2 * m],
                    op=mybir.AluOpType.subtract,
                )
                d3 = buf[:, 0:m].rearrange("p (r c) -> p c r", c=3)

                # Diagonal sums on the scalar engine (Square + accum_out),
                # overlapping with the DVE cross-products.
                if use_act:
                    for i in range(3):
                        sq = act_scr.tile([P, r], f32, tag="sq")
                        nc.scalar.activation(
                            out=sq[:],
                            in_=d3[:, i, :],
                            func=mybir.ActivationFunctionType.Square,
                            accum_out=acc_sq[:, t * 3 + i : t * 3 + i + 1],
                        )
                else:
                    for i in range(3):
                        sq = dve_scr.tile([P, r], f32, tag="pr")
                        nc.vector.scalar_tensor_tensor(
                            out=sq[:],
                            in0=d3[:, i, :],
                            scalar=1.0,
                            in1=d3[:, i, :],
                            op0=mybir.AluOpType.mult,
                            op1=mybir.AluOpType.mult,
                            accum_out=acc_sq[:, t * 3 + i : t * 3 + i + 1],
                        )
                # Cross sums: fused multiply+reduce on DVE
                # (scalar_tensor_tensor: out = (in0 * 1.0) * in1, accum = sum).
                for k, (i, j) in enumerate(_PAIRS):
                    pr = dve_scr.tile([P, r], f32, tag="pr")
                    nc.vector.scalar_tensor_tensor(
                        out=pr[:],
                        in0=d3[:, i, :],
                        scalar=1.0,
                        in1=d3[:, j, :],
                        op0=mybir.AluOpType.mult,
                        op1=mybir.AluOpType.mult,
                        accum_out=acc_cr[:, t * 3 + k : t * 3 + k + 1],
                    )

            nc.sync.dma_start(out=out[:, 0 : 3 * n_tiles], in_=acc_sq[:])
            nc.sync.dma_start(out=out[:, 3 * n_tiles : 6 * n_tiles], in_=acc_cr[:])

    nc.compile()
    return nc


def build_gram_kernel_raw(n_rows: int, n_tiles: int = 32, n_bufs: int = 24,
                          group: int = 4, skip_exit_barrier: bool = True):
    """Raw-Bacc variant: manual semaphores, no TileContext.

    Skips Tile's prologue/epilogue (drain + two all-engine EVSEM
    barriers, ~16 us) — the only sync needed is a three-semaphore chain:
    DMA loads (one HWDGE ring) -> DVE -> ACT.

    The ring of tile buffers lives in ONE SBUF tensor so the fused
    multiply-reduces can span `group` consecutive tiles with a single
    instruction (free-dim AP [group, r]) — amortizing the per-op fixed
    cost and the accumulator-drain, which keeps both compute engines
    well under the DMA pace.

    Input: pt [2, n_rows, 3] f32. Output: partials [128, 6 * n_groups]
    (same slot layout as build_gram_kernel, with n_groups slots).
    """
    assert n_tiles % group == 0 and n_bufs % group == 0
    assert n_rows % (P * n_tiles) == 0
    n_groups = n_tiles // group
    r = n_rows // (P * n_tiles)
    m = 3 * r
    f32 = mybir.dt.float32

    nc = bacc.Bacc("TRN2", target_bir_lowering=False, debug=False)
    pt = nc.dram_tensor("pt", [2, n_rows, 3], f32, kind="ExternalInput")
    out = nc.dram_tensor("partials", [P, 6 * n_groups], f32, kind="ExternalOutput")
    pt_v = pt[:].rearrange("w (t p r) c -> t p w (r c)", t=n_tiles, p=P)

    ring = nc.alloc_sbuf_tensor("ring", [P, n_bufs * 2 * m], f32).ap()

    def buf(t):
        s = t % n_bufs
        return ring[:, s * 2 * m : (s + 1) * 2 * m]

    def dgroup(g, i):
        # component i of the diff halves of tiles 4g..4g+3: [128, group, r]
        s0 = (g * group) % n_bufs
        w = ring[:, s0 * 2 * m : (s0 + group) * 2 * m]
        return w.rearrange("p (t w r c) -> p t w c r", t=group, w=2, c=3)[:, :, 0, i, :]

    acc_sq = nc.alloc_sbuf_tensor("acc_sq", [P, 3 * n_groups], f32).ap()
    acc_cr = nc.alloc_sbuf_tensor("acc_cr", [P, 3 * n_groups], f32).ap()
    # Rotated scratch (dead stores of the fused ops), 2 groups deep so each
    # group's single stale semaphore wait also covers the scratch WAW from
    # two groups back.
    pr_scrs = [
        nc.alloc_sbuf_tensor(f"pr_scr{k}", [P, group * r], f32).ap() for k in range(6)
    ]
    sq_scrs = [
        nc.alloc_sbuf_tensor(f"sq_scr{k}", [P, group * r], f32).ap() for k in range(6)
    ]

    # One DMA-completion semaphore per ring buffer: a single shared sem
    # would be unsound — each dma_start is split across 16 SDMA engines
    # whose sub-completions interleave across in-flight DMAs.
    dma_sems = [nc.alloc_semaphore(f"dma_sem{i}") for i in range(n_bufs)]
    out_sem = nc.alloc_semaphore("out_sem")
    dve_sem = nc.alloc_semaphore("dve_sem")
    act_sem = nc.alloc_semaphore("act_sem")

    # DVE emission order: subs run ahead; the grouped multiply-reduces for
    # group g are emitted after sub(4g+4) so their drain-wait on the last
    # sub of the group is already satisfied when it executes (DVE writes
    # drain asynchronously). Only the last group trails the final sub.
    dve_order = []
    for t in range(n_tiles):
        dve_order.append(("sub", t))
        if t % group == 0 and t >= group:
            # one sub of stagger after the group's last sub
            dve_order.append(("stt", t // group - 1))
    dve_order.append(("stt", n_groups - 1))
    sub_done, sttg_done = {}, {}
    v = 0
    for kind, x in dve_order:
        if kind == "sub":
            v += 1
            sub_done[x] = v
        else:
            v += 3
            sttg_done[x] = v

    # Output chunks: flush finished accumulator columns while later tiles
    # still stream, so the tail only waits on the last small chunk.
    chunk = max(1, n_groups // 2)
    chunks = [(c, min(c + chunk, n_groups)) for c in range(0, n_groups, chunk)]

    import contextlib

    @contextlib.contextmanager
    def _block():
        # no_gpsimd_drain=True emits per-engine drains explicitly and then a
        # sem-only all-engine butterfly. The butterfly only delays NEFF end
        # (outputs are already fenced by the sequencer's out_sem wait), so
        # optionally no-op it during Block.__exit__.
        with nc.Block(no_gpsimd_drain=True) as blk:
            try:
                yield blk
            finally:
                if skip_exit_barrier:
                    nc.all_engine_barrier = lambda **kw: None
        if skip_exit_barrier:
            del nc.all_engine_barrier  # restore class method

    with _block() as block:

        @block.sync
        def _(sync):
            for t in range(n_tiles):
                if t >= n_bufs:
                    # ring reuse: all consumers of the buffer's previous
                    # occupant (tile t - n_bufs) must be done
                    prev = t - n_bufs
                    sync.wait_ge(dve_sem, sttg_done[prev // group])
                    sync.wait_ge(act_sem, 3 * (prev // group + 1))
                sync.dma_start(
                    out=buf(t).rearrange("p (w m) -> p w m", w=2),
                    in_=pt_v[t],
                ).then_inc(dma_sems[t % n_bufs], 16)
            n_out = 0
            for lo, hi in chunks:
                sync.wait_ge(act_sem, 3 * hi)
                sync.dma_start(
                    out=out[:, 3 * lo : 3 * hi], in_=acc_sq[:, 3 * lo : 3 * hi]
                ).then_inc(out_sem, 16)
                sync.wait_ge(dve_sem, sttg_done[hi - 1])
                sync.dma_start(
                    out=out[:, 3 * (n_groups + lo) : 3 * (n_groups + hi)],
                    in_=acc_cr[:, 3 * lo : 3 * hi],
                ).then_inc(out_sem, 16)
                n_out += 32
            sync.wait_ge(out_sem, n_out)

        @block.vector
        def _(vector):
            for kind, x in dve_order:
                if kind == "sub":
                    b = buf(x)
                    vector.wait_ge(dma_sems[x % n_bufs], 16 * (x // n_bufs + 1))
                    vector.tensor_tensor(
                        out=b[:, 0:m],
                        in0=b[:, 0:m],
                        in1=b[:, m : 2 * m],
                        op=mybir.AluOpType.subtract,
                    ).then_inc(dve_sem, 1)
                else:
                    vector.wait_ge(dve_sem, sub_done[(x + 1) * group - 1])
                    for k, (i, j) in enumerate(_PAIRS):
                        vector.scalar_tensor_tensor(
                            out=pr_scrs[(x % 2) * 3 + k][:].rearrange(
                                "p (t r) -> p t r", t=group
                            ),
                            in0=dgroup(x, i),
                            scalar=1.0,
                            in1=dgroup(x, j),
                            op0=mybir.AluOpType.mult,
                            op1=mybir.AluOpType.mult,
                            accum_out=acc_cr[:, x * 3 + k : x * 3 + k + 1],
                        ).then_inc(dve_sem, 1)

        @block.scalar
        def _(scalar):
            for g in range(n_groups):
                scalar.wait_ge(dve_sem, sub_done[(g + 1) * group - 1])
                if g >= 2:
                    # scratch slot reuse from two groups back
                    scalar.wait_ge(act_sem, 3 * (g - 1))
                for i in range(3):
                    scalar.activation(
                        out=sq_scrs[(g % 2) * 3 + i][:].rearrange(
                            "p (t r) -> p t r", t=group
                        ),
                        in_=dgroup(g, i),
                        func=mybir.ActivationFunctionType.Square,
                        accum_out=acc_sq[:, g * 3 + i : g * 3 + i + 1],
                    ).then_inc(act_sem, 1)

    nc.compile()
    return nc

_NC_CACHE: dict[tuple, object] = {}


def _get_nc(n_rows: int, n_tiles: int, use_act: bool, raw: bool = False,
            group: int = 4):
    key = (n_rows, n_tiles, use_act, raw, group)
    if key not in _NC_CACHE:
        if raw:
            _NC_CACHE[key] = build_gram_kernel_raw(n_rows, n_tiles, group=group)
        else:
            _NC_CACHE[key] = build_gram_kernel(n_rows, n_tiles, use_act)
    return _NC_CACHE[key]


def gram_from_partials(partials: np.ndarray, n_tiles: int | None = None) -> np.ndarray:
    """[..., 128, 6*slots] partials -> full 3x3 Gram matrix (float64)."""
    slots = partials.shape[-1] // 6
    s = partials.astype(np.float64).reshape(-1, 6 * slots).sum(axis=0)
    sq = s[: 3 * slots].reshape(slots, 3).sum(axis=0)
    cr = s[3 * slots :].reshape(slots, 3).sum(axis=0)
    g = np.empty((3, 3), dtype=np.float64)
    g[0, 0], g[1, 1], g[2, 2] = sq
    for k, (i, j) in enumerate(_PAIRS):
        g[i, j] = g[j, i] = cr[k]
    return g


def run_device_partials(predictions: np.ndarray, targets: np.ndarray,
                        n_tiles: int = 4, use_act: bool = True,
                        raw: bool = False, group: int = 4, **run_kwargs):
    """Shard over N_CORES, run on device, return per-core partials + results."""
    b = predictions.shape[0]
    assert b % N_CORES == 0
    n_rows = b // N_CORES
    nc = _get_nc(n_rows, n_tiles, use_act, raw, group)
    preds = np.ascontiguousarray(predictions, dtype=np.float32).reshape(
        N_CORES, n_rows, 3
    )
    targs = np.ascontiguousarray(targets, dtype=np.float32).reshape(
        N_CORES, n_rows, 3
    )
    in_maps = [
        {"pt": np.stack([preds[c], targs[c]])} for c in range(N_CORES)
    ]
    res = run_bass_kernel_spmd(nc, in_maps, list(range(N_CORES)), **run_kwargs)
    partials = np.stack([r["partials"] for r in res.results])
    return partials, res


def kernel(predictions: np.ndarray, targets: np.ndarray, sigma: np.ndarray) -> np.ndarray:
    predictions = np.asarray(predictions, dtype=np.float32)
    targets = np.asarray(targets, dtype=np.float32)
    sigma64 = np.asarray(sigma, dtype=np.float64)

    partials, _ = run_device_partials(predictions, targets, n_tiles=32, raw=True)
    g = gram_from_partials(partials)

    sigma_inv = np.linalg.inv(sigma64)
    _, logdet = np.linalg.slogdet(sigma64)
    mean_mahal = float((sigma_inv * g).sum()) / predictions.shape[0]
    loss = abs(logdet + mean_mahal)
    return np.float32(loss)



# revision 3
# speedup vs baseline: 1.0827x; 1.0827x over previous
"""Trainium2 Bass kernel for CustomLossWithCovariance.

loss = abs(logdet(sigma) + mean_b[(p_b - t_b)^T sigma^{-1} (p_b - t_b)])

The device only ever needs sufficient statistics of d = pred - targ:

* sigma == c*I (the case this problem instantiates — setup_inputs builds
  sigma = (SIGMA_INIT + EPSILON) * eye(3)): the quadratic form reduces
  EXACTLY to ||d||^2 / c, so the device computes one scalar per
  partition: sum of d_i^2 over the whole shard (fast path,
  build_sumsq_kernel).  This is an algebraic identity, not an
  approximation — sigma_inv's off-diagonals are exact zeros.
* general sigma: the 3x3 Gram matrix G = sum_b d_b d_b^T suffices
  (mean_mahalanobis = <sigma_inv, G> / B); the device computes
  per-core partial pair-sums of G (build_gram_kernel_raw).

Host finishes with the tiny 3x3 algebra in float64.

Sharding: data-parallel over the batch across 8 NeuronCores; each core
streams a contiguous [B/8, 3] shard (24 MiB of f32), so the kernel is
HBM-bandwidth-bound (~60-64 us of streaming per core).  Host-side prep
re-packs each shard partition-major so every DMA descriptor is one
24 KiB contiguous run per partition.

Fast-path device kernel (raw Bacc, manual semaphores): per chunk of
K tiles ([128, K*1536] f32 = pred|targ interleaved per tile):
  - SP:  one dma_start per chunk (128 descriptors x K*6144 B)
  - DVE: one in-place tensor_tensor d = pred - targ  (unit-stride)
  - ACT: one Square activation with accum_out -> per-partition sum d^2
The final chunks are deliberately small (tiles [.., 2, 1, 1]) so the
post-stream compute tail is ~2 us instead of ~8.
"""

import numpy as np

import concourse.bass as bass
import concourse.bacc as bacc
import concourse.mybir as mybir
from concourse import tile
from concourse.bass_utils import run_bass_kernel_spmd

N_CORES = 8
B_FULL = 8388608
P = 128

_PAIRS = [(0, 1), (0, 2), (1, 2)]

# ---------------------------------------------------------------------------
# Fast path: sigma = c * I  ->  device computes sum over shard of d_i^2.
# ---------------------------------------------------------------------------

# Tiles of r rows per partition; m = 3*r f32 per tile per partition.
SS_R = 256
SS_M = 3 * SS_R                      # 768 f32 = 3 KiB
SS_TILES = 32                        # per core: 32*128*256 = 1,048,576 rows
# Chunk sizes (in tiles).  Bulk chunks of 4 amortize DMA/instruction
# overhead; the trailing 2/1/1 keep the post-stream compute tail short.
SS_CHUNKS = [4, 4, 4, 4, 4, 4, 4, 2, 1, 1]
SS_SLOTS = 7                         # ring slots, each sized for K=4


def build_sumsq_kernel(n_rows: int):
    """Per-core module: in pt [P, SS_TILES*2*SS_M] f32 (per partition, per
    tile: SS_M f32 of pred then SS_M f32 of targ, contiguous); out
    sums [P, n_chunks] f32 where col c = sum over chunk c of (p - t)^2.
    """
    assert n_rows == P * SS_R * SS_TILES
    assert sum(SS_CHUNKS) == SS_TILES
    n_chunks = len(SS_CHUNKS)
    kmax = max(SS_CHUNKS)
    m2 = 2 * SS_M                     # f32 per tile per partition (pred+targ)
    f32 = mybir.dt.float32

    nc = bacc.Bacc("TRN2", target_bir_lowering=False, debug=False)
    pt = nc.dram_tensor("pt", [P, SS_TILES * m2], f32, kind="ExternalInput")
    out = nc.dram_tensor("sums", [P, n_chunks], f32, kind="ExternalOutput")

    ring = nc.alloc_sbuf_tensor("ring", [P, SS_SLOTS * kmax * m2], f32).ap()
    acc = nc.alloc_sbuf_tensor("acc", [P, n_chunks], f32).ap()
    # Dead stores for the Square activations (2 rotating buffers; ACT is
    # serial so program order already fences reuse).
    scr = [nc.alloc_sbuf_tensor(f"scr{i}", [P, kmax * SS_M], f32).ap()
           for i in range(2)]

    dma_sems = [nc.alloc_semaphore(f"dma_sem{i}") for i in range(SS_SLOTS)]
    dve_sem = nc.alloc_semaphore("dve_sem")
    act_sem = nc.alloc_semaphore("act_sem")
    out_sem = nc.alloc_semaphore("out_sem")

    # chunk -> (tile offset, K)
    offs = []
    o = 0
    for k in SS_CHUNKS:
        offs.append((o, k))
        o += k

    def slot_ap(c: int):
        s = (c % SS_SLOTS) * kmax * m2
        k = SS_CHUNKS[c]
        return ring[:, s : s + k * m2]

    import contextlib

    @contextlib.contextmanager
    def _block():
        # Skip the exit-time all-engine sem barrier: outputs are already
        # fenced by the sequencer's out_sem wait, and the NEFF postamble
        # has its own barrier.
        with nc.Block(no_gpsimd_drain=True) as blk:
            try:
                yield blk
            finally:
                nc.all_engine_barrier = lambda **kw: None
        del nc.all_engine_barrier

    with _block() as block:

        @block.sync
        def _(sync):
            for c, (o, k) in enumerate(offs):
                if c >= SS_SLOTS:
                    # ACT is the last reader of the slot's previous chunk
                    sync.wait_ge(act_sem, c - SS_SLOTS + 1)
                sync.dma_start(
                    out=slot_ap(c), in_=pt[:, o * m2 : (o + k) * m2]
                ).then_inc(dma_sems[c % SS_SLOTS], 16)
            # Flush all but the last col while the tail chunks stream; the
            # final flush re-sends col n-2 (identical bytes) because a
            # single-column DRAM slice would be a non-contiguous AP.
            sync.wait_ge(act_sem, n_chunks - 1)
            sync.dma_start(
                out=out[:, 0 : n_chunks - 1], in_=acc[:, 0 : n_chunks - 1]
            ).then_inc(out_sem, 16)
            sync.wait_ge(act_sem, n_chunks)
            sync.dma_start(
                out=out[:, n_chunks - 2 : n_chunks],
                in_=acc[:, n_chunks - 2 : n_chunks],
            ).then_inc(out_sem, 16)
            sync.wait_ge(out_sem, 32)

        @block.vector
        def _(vector):
            for c, (o, k) in enumerate(offs):
                b = slot_ap(c)
                # [k, SS_M] view of the pred / targ halves of each tile
                pred = b.rearrange("p (t w m) -> p t w m", t=k, w=2)[:, :, 0, :]
                targ = b.rearrange("p (t w m) -> p t w m", t=k, w=2)[:, :, 1, :]
                vector.wait_ge(dma_sems[c % SS_SLOTS], 16 * (c // SS_SLOTS + 1))
                vector.tensor_tensor(
                    out=pred, in0=pred, in1=targ, op=mybir.AluOpType.subtract
                ).then_inc(dve_sem, 1)

        @block.scalar
        def _(scalar):
            for c, (o, k) in enumerate(offs):
                b = slot_ap(c)
                d = b.rearrange("p (t w m) -> p t w m", t=k, w=2)[:, :, 0, :]
                scalar.wait_ge(dve_sem, c + 1)
                scalar.activation(
                    out=scr[c % 2][:, : k * SS_M].rearrange(
                        "p (t m) -> p t m", t=k
                    ),
                    in_=d,
                    func=mybir.ActivationFunctionType.Square,
                    accum_out=acc[:, c : c + 1],
                ).then_inc(act_sem, 1)

    nc.compile()
    return nc


def _pack_shard(pred: np.ndarray, targ: np.ndarray) -> np.ndarray:
    """[n_rows, 3] pred/targ -> [P, SS_TILES, 2, SS_M] partition-major
    interleave so each (partition, tile) reads 6 KiB contiguous."""
    n_rows = pred.shape[0]
    assert n_rows == P * SS_TILES * SS_R
    arr = np.empty((P, SS_TILES, 2, SS_M), dtype=np.float32)
    arr[:, :, 0, :] = pred.reshape(P, SS_TILES, SS_M)
    arr[:, :, 1, :] = targ.reshape(P, SS_TILES, SS_M)
    return arr.reshape(P, SS_TILES * 2 * SS_M)


def run_device_sumsq(predictions: np.ndarray, targets: np.ndarray,
                     **run_kwargs):
    """Shard over N_CORES, run fast-path kernel, return per-core sums."""
    b = predictions.shape[0]
    assert b % N_CORES == 0
    n_rows = b // N_CORES
    nc = _get_nc_sumsq(n_rows)
    preds = np.ascontiguousarray(predictions, dtype=np.float32).reshape(
        N_CORES, n_rows, 3
    )
    targs = np.ascontiguousarray(targets, dtype=np.float32).reshape(
        N_CORES, n_rows, 3
    )
    in_maps = [
        {"pt": _pack_shard(preds[c], targs[c])} for c in range(N_CORES)
    ]
    res = run_bass_kernel_spmd(nc, in_maps, list(range(N_CORES)), **run_kwargs)
    sums = np.stack([r["sums"] for r in res.results])
    return sums, res


def _sigma_is_scalar_identity(sigma64: np.ndarray) -> bool:
    d = np.diag(sigma64)
    return (
        sigma64.shape == (3, 3)
        and np.count_nonzero(sigma64 - np.diag(d)) == 0
        and d[0] == d[1] == d[2]
        and d[0] > 0
    )


# ---------------------------------------------------------------------------
# General path: full 3x3 Gram matrix (kept from the baseline kernel).
# ---------------------------------------------------------------------------


def build_gram_kernel(n_rows: int, n_tiles: int, use_act: bool = True):
    """Build the per-core Bass module.

    Input: pt [2, n_rows, 3] f32 (pred stacked with targ)
    Output: partials [128, 6 * n_tiles] f32
        col t*3+i            : sum over this tile/partition of d_i^2
        col 3*n_tiles + t*3+k: sum of d_i*d_j for pair k in _PAIRS
    """
    assert n_rows % (P * n_tiles) == 0
    r = n_rows // (P * n_tiles)  # rows per partition per tile
    m = 3 * r                    # flat f32 elements per partition per tile
    f32 = mybir.dt.float32

    nc = bacc.Bacc("TRN2", target_bir_lowering=False, debug=False)
    pt = nc.dram_tensor("pt", [2, n_rows, 3], f32, kind="ExternalInput")
    out = nc.dram_tensor("partials", [P, 6 * n_tiles], f32, kind="ExternalOutput")

    pt_v = pt[:].rearrange("w (t p r) c -> t p w (r c)", t=n_tiles, p=P)

    with tile.TileContext(nc) as tc:
        with (
            tc.tile_pool(name="io", bufs=3) as io_pool,
            tc.tile_pool(name="dve_scr", bufs=2) as dve_scr,
            tc.tile_pool(name="act_scr", bufs=2) as act_scr,
            tc.tile_pool(name="acc", bufs=1) as acc_pool,
        ):
            acc_sq = acc_pool.tile([P, 3 * n_tiles], f32)
            acc_cr = acc_pool.tile([P, 3 * n_tiles], f32)

            for t in range(n_tiles):
                buf = io_pool.tile([P, 2 * m], f32, tag="buf")
                nc.sync.dma_start(
                    out=buf[:].rearrange("p (w m) -> p w m", w=2),
                    in_=pt_v[t],
                )

                nc.vector.tensor_tensor(
                    out=buf[:, 0:m],
                    in0=buf[:, 0:m],
                    in1=buf[:, m : 2 * m],
                    op=mybir.AluOpType.subtract,
                )
                d3 = buf[:, 0:m].rearrange("p (r c) -> p c r", c=3)

                if use_act:
                    for i in range(3):
                        sq = act_scr.tile([P, r], f32, tag="sq")
                        nc.scalar.activation(
                            out=sq[:],
                            in_=d3[:, i, :],
                            func=mybir.ActivationFunctionType.Square,
                            accum_out=acc_sq[:, t * 3 + i : t * 3 + i + 1],
                        )
                else:
                    for i in range(3):
                        sq = dve_scr.tile([P, r], f32, tag="pr")
                        nc.vector.scalar_tensor_tensor(
                            out=sq[:],
                            in0=d3[:, i, :],
                            scalar=1.0,
                            in1=d3[:, i, :],
                            op0=mybir.AluOpType.mult,
                            op1=mybir.AluOpType.mult,
                            accum_out=acc_sq[:, t * 3 + i : t * 3 + i + 1],
                        )
                for k, (i, j) in enumerate(_PAIRS):
                    pr = dve_scr.tile([P, r], f32, tag="pr")
                    nc.vector.scalar_tensor_tensor(
                        out=pr[:],
                        in0=d3[:, i, :],
                        scalar=1.0,
                        in1=d3[:, j, :],
                        op0=mybir.AluOpType.mult,
                        op1=mybir.AluOpType.mult,
                        accum_out=acc_cr[:, t * 3 + k : t * 3 + k + 1],
                    )

            nc.sync.dma_start(out=out[:, 0 : 3 * n_tiles], in_=acc_sq[:])
            nc.sync.dma_start(out=out[:, 3 * n_tiles : 6 * n_tiles], in_=acc_cr[:])

    nc.compile()
    return nc


_NC_CACHE: dict[tuple, object] = {}


def _get_nc_sumsq(n_rows: int):
    key = ("sumsq", n_rows)
    if key not in _NC_CACHE:
        _NC_CACHE[key] = build_sumsq_kernel(n_rows)
    return _NC_CACHE[key]


def _get_nc(n_rows: int, n_tiles: int, use_act: bool = True):
    key = (n_rows, n_tiles, use_act)
    if key not in _NC_CACHE:
        _NC_CACHE[key] = build_gram_kernel(n_rows, n_tiles, use_act)
    return _NC_CACHE[key]


def gram_from_partials(partials: np.ndarray, n_tiles: int | None = None) -> np.ndarray:
    """[..., 128, 6*slots] partials -> full 3x3 Gram matrix (float64)."""
    slots = partials.shape[-1] // 6
    s = partials.astype(np.float64).reshape(-1, 6 * slots).sum(axis=0)
    sq = s[: 3 * slots].reshape(slots, 3).sum(axis=0)
    cr = s[3 * slots :].reshape(slots, 3).sum(axis=0)
    g = np.empty((3, 3), dtype=np.float64)
    g[0, 0], g[1, 1], g[2, 2] = sq
    for k, (i, j) in enumerate(_PAIRS):
        g[i, j] = g[j, i] = cr[k]
    return g


def run_device_partials(predictions: np.ndarray, targets: np.ndarray,
                        n_tiles: int = 4, use_act: bool = True,
                        **run_kwargs):
    """Shard over N_CORES, run Gram kernel, return per-core partials."""
    b = predictions.shape[0]
    assert b % N_CORES == 0
    n_rows = b // N_CORES
    nc = _get_nc(n_rows, n_tiles, use_act)
    preds = np.ascontiguousarray(predictions, dtype=np.float32).reshape(
        N_CORES, n_rows, 3
    )
    targs = np.ascontiguousarray(targets, dtype=np.float32).reshape(
        N_CORES, n_rows, 3
    )
    in_maps = [
        {"pt": np.stack([preds[c], targs[c]])} for c in range(N_CORES)
    ]
    res = run_bass_kernel_spmd(nc, in_maps, list(range(N_CORES)), **run_kwargs)
    partials = np.stack([r["partials"] for r in res.results])
    return partials, res


# ---------------------------------------------------------------------------
# Entry point
# ---------------------------------------------------------------------------


def kernel(predictions: np.ndarray, targets: np.ndarray, sigma: np.ndarray) -> np.ndarray:
    predictions = np.asarray(predictions, dtype=np.float32)
    targets = np.asarray(targets, dtype=np.float32)
    sigma64 = np.asarray(sigma, dtype=np.float64)

    _, logdet = np.linalg.slogdet(sigma64)
    n = predictions.shape[0]

    if _sigma_is_scalar_identity(sigma64) and n == B_FULL:
        sums, _ = run_device_sumsq(predictions, targets)
        total = float(sums.astype(np.float64).sum())
        mean_mahal = total / (sigma64[0, 0] * n)
    else:
        partials, _ = run_device_partials(predictions, targets, n_tiles=32)
        g = gram_from_partials(partials)
        sigma_inv = np.linalg.inv(sigma64)
        mean_mahal = float((sigma_inv * g).sum()) / n

    loss = abs(logdet + mean_mahal)
    return np.float32(loss)
